# revision 5
# baseline (speedup 1.0000x reference)
"""Trainium2 Bass kernel for nn_ANIModelShare (ANI-style MoE routing).

Model (per atom n with species s(n)):
    h   = celu(aev @ Ws + bs)             # shared layer, D=1008 -> H=768
    h1  = celu(h @ W1[s] + b1[s])         # H -> 192
    h2  = celu(h1 @ W2[s] + b2[s])        # 192 -> 96
    e   = h2 @ W3[s] + b3[s]              # 96 -> 1
    energies[b] = sum over atoms of molecule b of e

Strategy:
  - Shard molecules across the 8 NeuronCores (64 molecules = 6144 atoms each).
    No collectives needed; outputs are concatenated on the host.
  - On the host, atoms are sorted by species per core and aev is pre-transposed
    to [D, atoms_sorted] (feature-major), so the whole MLP chain runs on device
    in "transposed" layout [features, atoms] with the contraction dim on
    partitions and no on-device gather/transpose anywhere.
  - Matmuls use float32r (rounded fp32) which runs at full PE rate for moving
    dims >= 256 (4x faster than plain fp32) with ~1e-4 relative rounding.
  - celu(z) = relu(z+b) + min(0.1*exp(10(z+b)), 0.1) - 0.1. The exp is one
    ScalarE activation (scale=10, bias=10b+ln 0.1 folded in), relu one DVE
    tensor_scalar, the combine one DVE scalar_tensor_tensor. The trailing
    -0.1 is folded into the next layer's bias on the host
    (b' = b - 0.1 * colsum(W)).
  - Per-molecule sums run on the PE: e is computed in natural [atom, 1] layout
    by using h2^T tiles as the stationary operand, then accumulated into a
    persistent [64, 1] PSUM tile against a host-built 0/1 molecule-assignment
    matrix (which also zeroes the padding columns).
"""
import math
import os

import numpy as np

import concourse.bass as bass
import concourse.mybir as mybir
import concourse.tile as tile
from concourse import bacc
from concourse.bass_utils import run_bass_kernel_spmd

F32 = mybir.dt.float32
F32R = mybir.dt.float32r
ALPHA = 0.1
LN_ALPHA = float(np.log(ALPHA))

B, A, D = 512, 96, 1008
H, H1, H2 = 768, 192, 96
S = 4
NCORES = 8
BS = B // NCORES          # molecules per core
NATOMS = BS * A           # atoms per core
KT0 = 8                   # stage-0 k tiles of 126 (8*126 = 1008)
K0 = D // KT0

LAST_EXEC_TIME_NS = None
LAST_RESULTS = None

_PROGRAM_CACHE = {}


def _chunk_sizes(C):
    """Split a species block of C columns (C % 128 == 0) into matmul chunks
    <= 512 wide, preferring >= 256 (float32r full-rate threshold)."""
    n128 = C // 128
    nparts = max(1, math.ceil(n128 / 4))
    base = n128 // nparts
    rem = n128 - base * nparts
    sizes = [(base + (1 if i < rem else 0)) * 128 for i in range(nparts)]
    assert sum(sizes) == C and all(sz <= 512 for sz in sizes)
    return sizes


def _build_program(C):
    """Build + compile the 8-core SPMD program for species capacity C."""
    chunks = []
    off = 0
    for sz in _chunk_sizes(C):
        chunks.append((off, sz))
        off += sz

    nc = bacc.Bacc("TRN2", target_bir_lowering=False, debug=False,
                   num_devices=NCORES)

    aevT = nc.declare_dram_parameter("aevT", [D, S * C], F32R, isOutput=False)
    masgn = nc.declare_dram_parameter("masgn", [S * C, BS], F32, isOutput=False)
    ws = nc.declare_dram_parameter("ws", [D, H], F32R, isOutput=False)
    w1 = nc.declare_dram_parameter("w1", [S, H, H1], F32R, isOutput=False)
    w2 = nc.declare_dram_parameter("w2", [S, H1, H2], F32R, isOutput=False)
    w3b = nc.declare_dram_parameter("w3b", [S, H2 + 1, 1], F32, isOutput=False)
    eb0 = nc.declare_dram_parameter("eb0", [H, 1], F32, isOutput=False)
    rb0 = nc.declare_dram_parameter("rb0", [H, 1], F32, isOutput=False)
    eb1 = nc.declare_dram_parameter("eb1", [S, H1, 1], F32, isOutput=False)
    rb1 = nc.declare_dram_parameter("rb1", [S, H1, 1], F32, isOutput=False)
    eb2 = nc.declare_dram_parameter("eb2", [S, H2, 1], F32, isOutput=False)
    rb2 = nc.declare_dram_parameter("rb2", [S, H2, 1], F32, isOutput=False)
    out = nc.declare_dram_parameter("out", [BS, 1], F32, isOutput=True)

    Exp = mybir.ActivationFunctionType.Exp
    add, amax, amin = (mybir.AluOpType.add, mybir.AluOpType.max,
                       mybir.AluOpType.min)

    n_red = S * sum(math.ceil(cb / 128) for _, cb in chunks)

    with tile.TileContext(nc) as tc:
        with (
            tc.tile_pool(name="wpool", bufs=1) as wpool,
            tc.tile_pool(name="apool", bufs=16) as apool,
            tc.tile_pool(name="hpool", bufs=12) as hpool,
            tc.tile_pool(name="h1pool", bufs=4) as h1pool,
            tc.tile_pool(name="h2pool", bufs=2) as h2pool,
            tc.tile_pool(name="spool", bufs=3) as spool,
            tc.tile_pool(name="esbp", bufs=2) as esbp,
            tc.tile_pool(name="mpool", bufs=4) as mpool,
            tc.tile_pool(name="psum0", bufs=2, space="PSUM") as psum0,
            tc.tile_pool(name="psum1", bufs=2, space="PSUM") as psum1,
            tc.tile_pool(name="psum2", bufs=1, space="PSUM") as psum2,
            tc.tile_pool(name="psume", bufs=1, space="PSUM") as psume,
            tc.tile_pool(name="psumE", bufs=1, space="PSUM") as psumE,
        ):
            # ---- persistent weights / biases -------------------------------
            ws_t = []
            for k in range(KT0):
                t = wpool.tile([K0, H], F32R, name=f"ws{k}", tag=f"ws{k}")
                nc.sync.dma_start(t[:], ws[k * K0:(k + 1) * K0, :])
                ws_t.append(t)
            w1_t = [[wpool.tile([128, H1], F32R, name=f"w1_{s}_{k}", tag=f"w1_{s}_{k}")
                     for k in range(6)] for s in range(S)]
            for s in range(S):
                for k in range(6):
                    nc.sync.dma_start(w1_t[s][k][:],
                                      w1[s, k * 128:(k + 1) * 128, :])
            w2_t = [[wpool.tile([128, H2], F32R, name=f"w2_{s}_0", tag=f"w2_{s}_0"),
                     wpool.tile([64, H2], F32R, name=f"w2_{s}_1", tag=f"w2_{s}_1")]
                    for s in range(S)]
            for s in range(S):
                nc.sync.dma_start(w2_t[s][0][:], w2[s, 0:128, :])
                nc.sync.dma_start(w2_t[s][1][:], w2[s, 128:192, :])
            w3b_t = [wpool.tile([H2 + 1, 1], F32, name=f"w3b{s}", tag=f"w3b{s}")
                     for s in range(S)]
            for s in range(S):
                nc.sync.dma_start(w3b_t[s][:], w3b[s, :, :])

            eb0_t, rb0_t = [], []
            for m in range(6):
                t = wpool.tile([128, 1], F32, name=f"eb0{m}", tag=f"eb0{m}")
                nc.sync.dma_start(t[:], eb0[m * 128:(m + 1) * 128, :])
                eb0_t.append(t)
                t = wpool.tile([128, 1], F32, name=f"rb0{m}", tag=f"rb0{m}")
                nc.sync.dma_start(t[:], rb0[m * 128:(m + 1) * 128, :])
                rb0_t.append(t)
            eb1_t = [[wpool.tile([128, 1], F32, name=f"eb1_{s}0", tag=f"eb1_{s}0"),
                      wpool.tile([64, 1], F32, name=f"eb1_{s}1", tag=f"eb1_{s}1")]
                     for s in range(S)]
            rb1_t = [[wpool.tile([128, 1], F32, name=f"rb1_{s}0", tag=f"rb1_{s}0"),
                      wpool.tile([64, 1], F32, name=f"rb1_{s}1", tag=f"rb1_{s}1")]
                     for s in range(S)]
            for s in range(S):
                nc.sync.dma_start(eb1_t[s][0][:], eb1[s, 0:128, :])
                nc.sync.dma_start(eb1_t[s][1][:], eb1[s, 128:192, :])
                nc.sync.dma_start(rb1_t[s][0][:], rb1[s, 0:128, :])
                nc.sync.dma_start(rb1_t[s][1][:], rb1[s, 128:192, :])
            eb2_t = [wpool.tile([H2, 1], F32, name=f"eb2{s}", tag=f"eb2{s}") for s in range(S)]
            rb2_t = [wpool.tile([H2, 1], F32, name=f"rb2{s}", tag=f"rb2{s}") for s in range(S)]
            for s in range(S):
                nc.sync.dma_start(eb2_t[s][:], eb2[s, :, :])
                nc.sync.dma_start(rb2_t[s][:], rb2[s, :, :])

            E_ps = psumE.tile([BS, 1], F32)

            red_i = 0
            for s in range(S):
                for coff, cb in chunks:
                    gcol = s * C + coff
                    # ---- load aev^T chunk ---------------------------------
                    at = []
                    for k in range(KT0):
                        t = apool.tile([K0, cb], F32R, name="aev", tag="aev")
                        nc.sync.dma_start(
                            t[:], aevT[k * K0:(k + 1) * K0, gcol:gcol + cb])
                        at.append(t)
                    # ---- stage 0: shared layer ----------------------------
                    ht = []
                    for m in range(6):
                        p0 = psum0.tile([128, cb], F32, name="p0", tag="p0")
                        for k in range(KT0):
                            nc.tensor.matmul(
                                p0[:], ws_t[k][:, m * 128:(m + 1) * 128],
                                at[k][:], start=(k == 0), stop=(k == KT0 - 1))
                        et = spool.tile([128, cb], F32, name="escr", tag="escr")
                        nc.scalar.activation(et[:], p0[:], Exp,
                                             bias=eb0_t[m][:], scale=10.0)
                        h = hpool.tile([128, cb], F32R, name="h", tag="h")
                        nc.vector.tensor_scalar(h[:], p0[:], rb0_t[m][:], 0.0,
                                                add, amax)
                        nc.vector.scalar_tensor_tensor(h[:], et[:], ALPHA,
                                                       h[:], amin, add)
                        ht.append(h)
                    # ---- stage 1 ------------------------------------------
                    h1t = []
                    for m1, msz in ((0, 128), (1, 64)):
                        p1 = psum1.tile([msz, cb], F32, name="p1", tag="p1")
                        for k in range(6):
                            nc.tensor.matmul(
                                p1[:],
                                w1_t[s][k][:, m1 * 128:m1 * 128 + msz],
                                ht[k][:], start=(k == 0), stop=(k == 5))
                        et = spool.tile([msz, cb], F32, name="escr", tag="escr")
                        nc.scalar.activation(et[:], p1[:], Exp,
                                             bias=eb1_t[s][m1][:], scale=10.0)
                        hh = h1pool.tile([msz, cb], F32R, name="h1", tag="h1")
                        nc.vector.tensor_scalar(hh[:], p1[:], rb1_t[s][m1][:],
                                                0.0, add, amax)
                        nc.vector.scalar_tensor_tensor(hh[:], et[:], ALPHA,
                                                       hh[:], amin, add)
                        h1t.append(hh)
                    # ---- stage 2 ------------------------------------------
                    p2 = psum2.tile([H2, cb], F32, name="p2", tag="p2")
                    nc.tensor.matmul(p2[:], w2_t[s][0][:], h1t[0][:],
                                     start=True, stop=False)
                    nc.tensor.matmul(p2[:], w2_t[s][1][:], h1t[1][:],
                                     start=False, stop=True)
                    et = spool.tile([H2, cb], F32, name="escr", tag="escr")
                    nc.scalar.activation(et[:], p2[:], Exp,
                                         bias=eb2_t[s][:], scale=10.0)
                    h2 = h2pool.tile([H2 + 1, cb], F32, name="h2", tag="h2")
                    nc.vector.tensor_scalar(h2[0:H2, :], p2[:], rb2_t[s][:],
                                            0.0, add, amax)
                    nc.vector.scalar_tensor_tensor(h2[0:H2, :], et[:], ALPHA,
                                                   h2[0:H2, :], amin, add)
                    nc.gpsimd.memset(h2[H2:H2 + 1, :], 1.0)
                    # ---- stage 3: per-atom energy, natural layout ---------
                    na = math.ceil(cb / 128)
                    e_ps = psume.tile([128, na], F32, name="eps", tag="eps")
                    for a in range(na):
                        sub = min(128, cb - a * 128)
                        nc.tensor.matmul(
                            e_ps[0:sub, a:a + 1],
                            h2[:, a * 128:a * 128 + sub],
                            w3b_t[s][:], start=True, stop=True)
                    e_sb = esbp.tile([128, na], F32, name="esb", tag="esb")
                    nc.vector.tensor_copy(e_sb[:], e_ps[:])
                    # ---- molecule-sum accumulate --------------------------
                    for a in range(na):
                        sub = min(128, cb - a * 128)
                        mt = mpool.tile([sub, BS], F32, name="mas", tag="mas")
                        r0 = gcol + a * 128
                        nc.sync.dma_start(mt[:], masgn[r0:r0 + sub, :])
                        nc.tensor.matmul(E_ps[:], mt[:],
                                         e_sb[0:sub, a:a + 1],
                                         start=(red_i == 0),
                                         stop=(red_i == n_red - 1))
                        red_i += 1

            E_sb = wpool.tile([BS, 1], F32, name="esbout", tag="esbout")
            nc.vector.tensor_copy(E_sb[:], E_ps[:])
            nc.sync.dma_start(out[:], E_sb[:])

    nc.compile()
    return nc


def _install_ntff_hook():
    """Register the axon NTFF profile hook (missing from this image's antenv)
    so run_bass_kernel_spmd(trace=True) can return exec_time_ns."""
    import contextlib
    import ctypes
    import sys
    import types

    if "antenv.axon_hooks" in sys.modules:
        return
    lib = ctypes.CDLL("/opt/axon/libaxon_pjrt.so")
    if not hasattr(lib, "axon_start_nrt_profile"):
        raise RuntimeError("libaxon_pjrt.so lacks axon_start_nrt_profile")
    lib.axon_start_nrt_profile.argtypes = [ctypes.POINTER(ctypes.c_int64),
                                           ctypes.c_size_t]
    lib.axon_start_nrt_profile.restype = ctypes.c_int64
    lib.axon_stop_nrt_profile.argtypes = [ctypes.c_char_p]
    lib.axon_stop_nrt_profile.restype = ctypes.c_int64

    @contextlib.contextmanager
    def _hook(output_dir, device_ids):
        import jax

        jax.devices()
        if device_ids:
            ids = (ctypes.c_int64 * len(device_ids))(*device_ids)
            rc = lib.axon_start_nrt_profile(ids, len(device_ids))
        else:
            rc = lib.axon_start_nrt_profile(None, 0)
        if rc != 0:
            raise RuntimeError(f"axon_start_nrt_profile rc={rc}")
        try:
            yield
        finally:
            n = lib.axon_stop_nrt_profile(str(output_dir).encode())
            if n <= 0:
                print(f"ntff profile: rc={n} for {output_dir}", file=sys.stderr)

    _state = {"hook": _hook}
    mod = types.ModuleType("antenv.axon_hooks")
    mod.get_axon_ntff_profile_hook = lambda: _state["hook"]
    mod.set_axon_ntff_profile_hook = lambda h: _state.__setitem__("hook", h)
    sys.modules["antenv.axon_hooks"] = mod
    import antenv

    antenv.axon_hooks = mod


def kernel(**inputs):
    global LAST_EXEC_TIME_NS, LAST_RESULTS
    species = np.asarray(inputs["species"]).astype(np.int64)
    aev = np.ascontiguousarray(np.asarray(inputs["aev"], dtype=np.float32))
    Ws = np.asarray(inputs["Ws"], dtype=np.float32)
    bs = np.asarray(inputs["bs"], dtype=np.float32)
    W1 = np.asarray(inputs["W1"], dtype=np.float32)
    b1 = np.asarray(inputs["b1"], dtype=np.float32)
    W2 = np.asarray(inputs["W2"], dtype=np.float32)
    b2 = np.asarray(inputs["b2"], dtype=np.float32)
    W3 = np.asarray(inputs["W3"], dtype=np.float32)
    b3 = np.asarray(inputs["b3"], dtype=np.float32)

    assert species.shape == (B, A) and aev.shape == (B, A, D)

    # ---- host-side routing: per-core species sort -------------------------
    sp = species.reshape(B, A)
    core_idx = []          # per core: list of 4 index arrays (into 0..NATOMS)
    maxcnt = 1
    for c in range(NCORES):
        spc = sp[c * BS:(c + 1) * BS].reshape(-1)
        spc_cl = np.clip(spc, 0, S - 1)
        idxs = [np.nonzero((spc >= 0) & (spc_cl == s))[0] for s in range(S)]
        core_idx.append(idxs)
        maxcnt = max(maxcnt, max(len(ix) for ix in idxs))
    C = ((maxcnt + 127) // 128) * 128

    if C not in _PROGRAM_CACHE:
        _PROGRAM_CACHE[C] = _build_program(C)
    nc = _PROGRAM_CACHE[C]

    # ---- host-side bias precompute (fold -0.1 celu offset forward) --------
    b1p = b1 - ALPHA * W1.sum(axis=1)                      # [S, H1]
    b2p = b2 - ALPHA * W2.sum(axis=1)                      # [S, H2]
    b3p = b3 - ALPHA * W3.sum(axis=1)                      # [S, 1]
    shared = {
        "ws": np.ascontiguousarray(Ws),
        "w1": np.ascontiguousarray(W1),
        "w2": np.ascontiguousarray(W2),
        "w3b": np.ascontiguousarray(
            np.concatenate([W3, b3p[:, None, :]], axis=1)),  # [S, 97, 1]
        "eb0": np.ascontiguousarray((10.0 * bs + LN_ALPHA)[:, None]),
        "rb0": np.ascontiguousarray(bs[:, None]),
        "eb1": np.ascontiguousarray((10.0 * b1p + LN_ALPHA)[..., None]),
        "rb1": np.ascontiguousarray(b1p[..., None]),
        "eb2": np.ascontiguousarray((10.0 * b2p + LN_ALPHA)[..., None]),
        "rb2": np.ascontiguousarray(b2p[..., None]),
    }

    in_maps = []
    for c in range(NCORES):
        aev_c = aev[c * BS:(c + 1) * BS].reshape(NATOMS, D)
        aevT = np.zeros((D, S * C), dtype=np.float32)
        mas = np.zeros((S * C, BS), dtype=np.float32)
        for s in range(S):
            ix = core_idx[c][s]
            n = len(ix)
            if n:
                aevT[:, s * C:s * C + n] = aev_c[ix].T
                mas[s * C + np.arange(n), ix // A] = 1.0
        m = {"aevT": aevT, "masgn": mas}
        m.update(shared)
        in_maps.append(m)

    trace = os.environ.get("BASS_KERNEL_TRACE", "") == "1"
    if trace:
        _install_ntff_hook()
    res = run_bass_kernel_spmd(nc, in_maps, core_ids=list(range(NCORES)),
                               trace=trace,
                               trace_cores=list(range(NCORES)) if trace else None)
    LAST_EXEC_TIME_NS = res.exec_time_ns
    LAST_RESULTS = res
    return np.concatenate(
        [res.results[c]["out"][:, 0] for c in range(NCORES)]).astype(np.float32)


# revision 7
# speedup vs baseline: 1.5142x; 1.5142x over previous
"""Trainium2 Bass kernel for nn_ANIModelShare (ANI-style MoE routing).

Model (per atom n with species s(n)):
    h   = celu(aev @ Ws + bs)             # shared layer, D=1008 -> H=768
    h1  = celu(h @ W1[s] + b1[s])         # H -> 192
    h2  = celu(h1 @ W2[s] + b2[s])        # 192 -> 96
    e   = h2 @ W3[s] + b3[s]              # 96 -> 1
    energies[b] = sum over atoms of molecule b of e

Strategy:
  - Shard molecules across the 8 NeuronCores (64 molecules = 6144 atoms each).
    No collectives needed; outputs are concatenated on the host.
  - On the host, atoms are sorted by species per core and aev is pre-transposed
    to [D, atoms_sorted] (feature-major), so the whole MLP chain runs on device
    in "transposed" layout [features, atoms] with the contraction dim on
    partitions and no on-device gather/transpose anywhere.
  - Matmuls use float32r (rounded fp32) which runs at full PE rate for moving
    dims >= 256 (4x faster than plain fp32) with ~1e-4 relative rounding.
  - celu(z) = relu(z+b) + min(0.1*exp(10(z+b)), 0.1) - 0.1. The exp is one
    ScalarE activation (scale=10, bias=10b+ln 0.1 folded in), relu one
    tensor_scalar, the combine one scalar_tensor_tensor. The trailing -0.1 is
    folded into the next layer's bias on the host (b' = b - 0.1*colsum(W)).
  - Per-molecule sums run on the PE: e is computed in natural [atom, 1] layout
    by using h2^T tiles as the stationary operand, then accumulated into a
    persistent [64, 1] PSUM tile against a host-built 0/1 molecule-assignment
    matrix (which also zeroes the padding columns). These tiny matmuls run in
    plain fp32 (fp32r has dst-pattern ISA restrictions at free size 1).
  - All weights/masks are DMAed in a handful of large packed transfers, and
    the emission is software-pipelined (stage k consumes the chunk produced
    3 iterations earlier) so the PE never stalls on activation epilogues and
    the HAM clock gate stays warm.
"""
import math
import os
from collections import deque

import numpy as np

import concourse.mybir as mybir
import concourse.tile as tile
from concourse import bacc
from concourse.bass_utils import run_bass_kernel_spmd

F32 = mybir.dt.float32
F32R = mybir.dt.float32r
ALPHA = 0.1
LN_ALPHA = float(np.log(ALPHA))

B, A, D = 512, 96, 1008
H, H1, H2 = 768, 192, 96
S = 4
NCORES = 8
BS = B // NCORES          # molecules per core
NATOMS = BS * A           # atoms per core
KT0 = 8                   # stage-0 k tiles of 126 (8*126 = 1008)
K0 = D // KT0

LAST_EXEC_TIME_NS = None
LAST_RESULTS = None

_PROGRAM_CACHE = {}


def _chunk_sizes(C):
    """Split a species block of C columns (C % 128 == 0) into matmul chunks
    <= 512 wide, preferring >= 256 (float32r full-rate threshold)."""
    n128 = C // 128
    nparts = max(1, math.ceil(n128 / 4))
    base = n128 // nparts
    rem = n128 - base * nparts
    sizes = [(base + (1 if i < rem else 0)) * 128 for i in range(nparts)]
    assert sum(sizes) == C and all(sz <= 512 for sz in sizes)
    return sizes


def _build_program(C):
    """Build + compile the 8-core SPMD program for species capacity C."""
    chunks = []
    off = 0
    for sz in _chunk_sizes(C):
        chunks.append((off, sz))
        off += sz
    iters = [(s, coff, cb) for s in range(S) for (coff, cb) in chunks]
    n_red = sum(math.ceil(cb / 128) for _, _, cb in iters)
    SC = S * C

    nc = bacc.Bacc("TRN2", target_bir_lowering=False, debug=False,
                   num_devices=NCORES)

    # aevT packed [126, 8, S*C]: aevTp[p, k, col] = feature (k*126+p) of col
    aevT = nc.declare_dram_parameter("aevT", [K0, KT0, SC], F32R, isOutput=False)
    # masgn packed [128, ntile, 64]: row p of subtile t = sorted atom t*128+p
    ntile = SC // 128
    masgn = nc.declare_dram_parameter("masgn", [128, ntile, BS], F32,
                                      isOutput=False)
    # ws packed [126, 8, 768]
    ws = nc.declare_dram_parameter("ws", [K0, KT0, H], F32R, isOutput=False)
    # w1 packed [128, S, 6, 192]
    w1 = nc.declare_dram_parameter("w1", [128, S, 6, H1], F32R, isOutput=False)
    w2a = nc.declare_dram_parameter("w2a", [128, S, H2], F32R, isOutput=False)
    w2b = nc.declare_dram_parameter("w2b", [64, S, H2], F32R, isOutput=False)
    w3b = nc.declare_dram_parameter("w3b", [H2 + 1, S], F32, isOutput=False)
    # biases packed [128, 36]: cols 0-5 eb0, 6-11 rb0, 12-19 eb1 (2/species),
    # 20-27 rb1, 28-31 eb2, 32-35 rb2
    biases = nc.declare_dram_parameter("biases", [128, 36], F32, isOutput=False)
    out = nc.declare_dram_parameter("out", [BS, 1], F32, isOutput=True)

    Exp = mybir.ActivationFunctionType.Exp
    add, amax, amin = (mybir.AluOpType.add, mybir.AluOpType.max,
                       mybir.AluOpType.min)

    with tile.TileContext(nc) as tc:
        with (
            tc.tile_pool(name="wpool", bufs=1) as wpool,
            tc.tile_pool(name="apool", bufs=2) as apool,
            tc.tile_pool(name="hpool", bufs=12) as hpool,
            tc.tile_pool(name="h1pool", bufs=4) as h1pool,
            tc.tile_pool(name="h2pool", bufs=2) as h2pool,
            tc.tile_pool(name="spool", bufs=4) as spool,
            tc.tile_pool(name="esbp", bufs=2) as esbp,
            tc.tile_pool(name="psum0", bufs=3, space="PSUM") as psum0,
            tc.tile_pool(name="psum1", bufs=2, space="PSUM") as psum1,
            tc.tile_pool(name="psum2", bufs=1, space="PSUM") as psum2,
            tc.tile_pool(name="psume", bufs=1, space="PSUM") as psume,
            tc.tile_pool(name="psumE", bufs=1, space="PSUM") as psumE,
        ):
            # ---- stage-0 weights first (unblock first matmuls) -------------
            ws_t = wpool.tile([K0, KT0, H], F32R, name="wst", tag="wst")
            nc.sync.dma_start(ws_t[:], ws[:])
            bias_t = wpool.tile([128, 36], F32, name="biast", tag="biast")
            nc.sync.dma_start(bias_t[:], biases[:])

            eb0_t = [bias_t[:, m:m + 1] for m in range(6)]
            rb0_t = [bias_t[:, 6 + m:7 + m] for m in range(6)]
            eb1_t = [[bias_t[:, 12 + 2 * s:13 + 2 * s],
                      bias_t[0:64, 13 + 2 * s:14 + 2 * s]] for s in range(S)]
            rb1_t = [[bias_t[:, 20 + 2 * s:21 + 2 * s],
                      bias_t[0:64, 21 + 2 * s:22 + 2 * s]] for s in range(S)]
            eb2_t = [bias_t[0:H2, 28 + s:29 + s] for s in range(S)]
            rb2_t = [bias_t[0:H2, 32 + s:33 + s] for s in range(S)]

            # declared lazily after the first chunk's stage-0 is emitted so
            # their DMAs don't delay the first matmuls
            late = {}

            E_ps = psumE.tile([BS, 1], F32)
            red_i = 0

            q1 = deque()   # (s, cb, ht)
            q2 = deque()   # (s, cb, h1t)
            q3 = deque()   # (s, cb, h2)

            def emit_stage0(it):
                s, coff, cb = it
                gcol = s * C + coff
                at = apool.tile([K0, KT0, cb], F32R, name="aev", tag="aev")
                nc.sync.dma_start(at[:], aevT[:, :, gcol:gcol + cb])
                ht = []
                for m in range(6):
                    p0 = psum0.tile([128, cb], F32, name="p0", tag="p0")
                    for k in range(KT0):
                        nc.tensor.matmul(
                            p0[:], ws_t[:, k, m * 128:(m + 1) * 128],
                            at[:, k, :], start=(k == 0), stop=(k == KT0 - 1))
                    et = spool.tile([128, cb], F32, name="escr", tag="escr")
                    nc.scalar.activation(et[:], p0[:], Exp,
                                         bias=eb0_t[m], scale=10.0)
                    h = hpool.tile([128, cb], F32R, name="h", tag="h")
                    nc.any.tensor_scalar(h[:], p0[:], rb0_t[m], 0.0, add, amax)
                    nc.vector.scalar_tensor_tensor(h[:], et[:], ALPHA, h[:],
                                                amin, add)
                    ht.append(h)
                q1.append((s, cb, ht))

            def emit_stage1():
                s, cb, ht = q1.popleft()
                w1_t = late["w1_t"]
                h1t = []
                for m1, msz in ((0, 128), (1, 64)):
                    p1 = psum1.tile([msz, cb], F32, name="p1", tag="p1")
                    for k in range(6):
                        nc.tensor.matmul(
                            p1[:], w1_t[:, s, k, m1 * 128:m1 * 128 + msz],
                            ht[k][:], start=(k == 0), stop=(k == 5))
                    et = spool.tile([msz, cb], F32, name="escr", tag="escr")
                    nc.scalar.activation(et[:], p1[:], Exp,
                                         bias=eb1_t[s][m1], scale=10.0)
                    hh = h1pool.tile([msz, cb], F32R, name="h1", tag="h1")
                    nc.any.tensor_scalar(hh[:], p1[:], rb1_t[s][m1], 0.0,
                                         add, amax)
                    nc.vector.scalar_tensor_tensor(hh[:], et[:], ALPHA, hh[:],
                                                amin, add)
                    h1t.append(hh)
                q2.append((s, cb, h1t))

            def emit_stage2():
                s, cb, h1t = q2.popleft()
                p2 = psum2.tile([H2, cb], F32, name="p2", tag="p2")
                nc.tensor.matmul(p2[:], late["w2a_t"][:, s, :], h1t[0][:],
                                 start=True, stop=False)
                nc.tensor.matmul(p2[:], late["w2b_t"][:, s, :], h1t[1][:],
                                 start=False, stop=True)
                et = spool.tile([H2, cb], F32, name="escr", tag="escr")
                nc.scalar.activation(et[:], p2[:], Exp,
                                     bias=eb2_t[s], scale=10.0)
                h2 = h2pool.tile([H2 + 1, cb], F32, name="h2", tag="h2")
                nc.any.tensor_scalar(h2[0:H2, :], p2[:], rb2_t[s], 0.0,
                                     add, amax)
                nc.vector.scalar_tensor_tensor(h2[0:H2, :], et[:], ALPHA,
                                            h2[0:H2, :], amin, add)
                nc.vector.memset(h2[H2:H2 + 1, :], 1.0)
                q3.append((s, cb, h2))

            def emit_stage3(it):
                nonlocal red_i
                s, coff, cb = it
                _s, _cb, h2 = q3.popleft()
                assert _s == s and _cb == cb
                gcol = s * C + coff
                na = math.ceil(cb / 128)
                e_ps = psume.tile([128, na], F32, name="eps", tag="eps")
                for a in range(na):
                    sub = min(128, cb - a * 128)
                    nc.tensor.matmul(
                        e_ps[0:sub, a:a + 1],
                        h2[:, a * 128:a * 128 + sub],
                        late["w3b_t"][:, s:s + 1], start=True, stop=True)
                e_sb = esbp.tile([128, na], F32, name="esb", tag="esb")
                nc.any.tensor_copy(e_sb[:], e_ps[:])
                t0 = gcol // 128
                for a in range(na):
                    sub = min(128, cb - a * 128)
                    nc.tensor.matmul(E_ps[:],
                                     late["mas_t"][0:sub, t0 + a, :],
                                     e_sb[0:sub, a:a + 1],
                                     start=(red_i == 0),
                                     stop=(red_i == n_red - 1))
                    red_i += 1

            n = len(iters)
            for i in range(n + 3):
                if i < n:
                    emit_stage0(iters[i])
                if i == 0:
                    # late weights: DMAs issued after the first stage-0 so
                    # the PE can start as soon as ws + first aev chunk land
                    t = wpool.tile([128, S, 6, H1], F32R, name="w1t", tag="w1t")
                    nc.sync.dma_start(t[:], w1[:])
                    late["w1_t"] = t
                    t = wpool.tile([128, S, H2], F32R, name="w2at", tag="w2at")
                    nc.sync.dma_start(t[:], w2a[:])
                    late["w2a_t"] = t
                    t = wpool.tile([64, S, H2], F32R, name="w2bt", tag="w2bt")
                    nc.sync.dma_start(t[:], w2b[:])
                    late["w2b_t"] = t
                    t = wpool.tile([H2 + 1, S], F32, name="w3bt", tag="w3bt")
                    nc.sync.dma_start(t[:], w3b[:])
                    late["w3b_t"] = t
                    t = wpool.tile([128, ntile, BS], F32, name="mast",
                                   tag="mast")
                    nc.sync.dma_start(t[:], masgn[:])
                    late["mas_t"] = t
                if i >= 1 and q1:
                    emit_stage1()
                if i >= 2 and q2:
                    emit_stage2()
                if i >= 3 and q3:
                    emit_stage3(iters[i - 3])

            E_sb = wpool.tile([BS, 1], F32, name="esbout", tag="esbout")
            nc.vector.tensor_copy(E_sb[:], E_ps[:])
            nc.sync.dma_start(out[:], E_sb[:])

    nc.compile()
    return nc, C


def _install_ntff_hook():
    """Register the axon NTFF profile hook (missing from this image's antenv)
    so run_bass_kernel_spmd(trace=True) can return exec_time_ns."""
    import contextlib
    import ctypes
    import sys
    import types

    if "antenv.axon_hooks" in sys.modules:
        return
    lib = ctypes.CDLL("/opt/axon/libaxon_pjrt.so")
    if not hasattr(lib, "axon_start_nrt_profile"):
        raise RuntimeError("libaxon_pjrt.so lacks axon_start_nrt_profile")
    lib.axon_start_nrt_profile.argtypes = [ctypes.POINTER(ctypes.c_int64),
                                           ctypes.c_size_t]
    lib.axon_start_nrt_profile.restype = ctypes.c_int64
    lib.axon_stop_nrt_profile.argtypes = [ctypes.c_char_p]
    lib.axon_stop_nrt_profile.restype = ctypes.c_int64

    @contextlib.contextmanager
    def _hook(output_dir, device_ids):
        import jax

        jax.devices()
        if device_ids:
            ids = (ctypes.c_int64 * len(device_ids))(*device_ids)
            rc = lib.axon_start_nrt_profile(ids, len(device_ids))
        else:
            rc = lib.axon_start_nrt_profile(None, 0)
        if rc != 0:
            raise RuntimeError(f"axon_start_nrt_profile rc={rc}")
        try:
            yield
        finally:
            n = lib.axon_stop_nrt_profile(str(output_dir).encode())
            if n <= 0:
                print(f"ntff profile: rc={n} for {output_dir}", file=sys.stderr)

    _state = {"hook": _hook}
    mod = types.ModuleType("antenv.axon_hooks")
    mod.get_axon_ntff_profile_hook = lambda: _state["hook"]
    mod.set_axon_ntff_profile_hook = lambda h: _state.__setitem__("hook", h)
    sys.modules["antenv.axon_hooks"] = mod
    import antenv

    antenv.axon_hooks = mod


def kernel(**inputs):
    global LAST_EXEC_TIME_NS, LAST_RESULTS
    species = np.asarray(inputs["species"]).astype(np.int64)
    aev = np.ascontiguousarray(np.asarray(inputs["aev"], dtype=np.float32))
    Ws = np.asarray(inputs["Ws"], dtype=np.float32)
    bs = np.asarray(inputs["bs"], dtype=np.float32)
    W1 = np.asarray(inputs["W1"], dtype=np.float32)
    b1 = np.asarray(inputs["b1"], dtype=np.float32)
    W2 = np.asarray(inputs["W2"], dtype=np.float32)
    b2 = np.asarray(inputs["b2"], dtype=np.float32)
    W3 = np.asarray(inputs["W3"], dtype=np.float32)
    b3 = np.asarray(inputs["b3"], dtype=np.float32)

    assert species.shape == (B, A) and aev.shape == (B, A, D)

    # ---- host-side routing: per-core species sort -------------------------
    sp = species.reshape(B, A)
    core_idx = []
    maxcnt = 1
    for c in range(NCORES):
        spc = sp[c * BS:(c + 1) * BS].reshape(-1)
        spc_cl = np.clip(spc, 0, S - 1)
        idxs = [np.nonzero((spc >= 0) & (spc_cl == s))[0] for s in range(S)]
        core_idx.append(idxs)
        maxcnt = max(maxcnt, max(len(ix) for ix in idxs))
    C = ((maxcnt + 127) // 128) * 128

    if C not in _PROGRAM_CACHE:
        _PROGRAM_CACHE[C] = _build_program(C)
    nc, _ = _PROGRAM_CACHE[C]
    SC = S * C

    # ---- host-side bias precompute (fold -0.1 celu offset forward) --------
    b1p = b1 - ALPHA * W1.sum(axis=1)                      # [S, H1]
    b2p = b2 - ALPHA * W2.sum(axis=1)                      # [S, H2]
    b3p = b3 - ALPHA * W3.sum(axis=1)                      # [S, 1]

    biases = np.zeros((128, 36), dtype=np.float32)
    eb0 = 10.0 * bs + LN_ALPHA
    eb1 = 10.0 * b1p + LN_ALPHA
    eb2 = 10.0 * b2p + LN_ALPHA
    for m in range(6):
        biases[:, m] = eb0[m * 128:(m + 1) * 128]
        biases[:, 6 + m] = bs[m * 128:(m + 1) * 128]
    for s in range(S):
        biases[:, 12 + 2 * s] = eb1[s, 0:128]
        biases[0:64, 13 + 2 * s] = eb1[s, 128:192]
        biases[:, 20 + 2 * s] = b1p[s, 0:128]
        biases[0:64, 21 + 2 * s] = b1p[s, 128:192]
        biases[0:H2, 28 + s] = eb2[s]
        biases[0:H2, 32 + s] = b2p[s]

    shared = {
        "ws": np.ascontiguousarray(
            Ws.reshape(KT0, K0, H).transpose(1, 0, 2)),          # [126,8,768]
        "w1": np.ascontiguousarray(
            W1.reshape(S, 6, 128, H1).transpose(2, 0, 1, 3)),    # [128,S,6,192]
        "w2a": np.ascontiguousarray(W2[:, 0:128, :].transpose(1, 0, 2)),
        "w2b": np.ascontiguousarray(W2[:, 128:192, :].transpose(1, 0, 2)),
        "w3b": np.ascontiguousarray(
            np.concatenate([W3, b3p[:, None, :]], axis=1)[:, :, 0].T),
        "biases": biases,
    }

    in_maps = []
    for c in range(NCORES):
        aev_c = aev[c * BS:(c + 1) * BS].reshape(NATOMS, D)
        aevT = np.zeros((D, SC), dtype=np.float32)
        mas = np.zeros((SC, BS), dtype=np.float32)
        for s in range(S):
            ix = core_idx[c][s]
            n = len(ix)
            if n:
                aevT[:, s * C:s * C + n] = aev_c[ix].T
                mas[s * C + np.arange(n), ix // A] = 1.0
        m = {
            "aevT": np.ascontiguousarray(
                aevT.reshape(KT0, K0, SC).transpose(1, 0, 2)),
            "masgn": np.ascontiguousarray(
                mas.reshape(SC // 128, 128, BS).transpose(1, 0, 2)),
        }
        m.update(shared)
        in_maps.append(m)

    trace = os.environ.get("BASS_KERNEL_TRACE", "") == "1"
    if trace:
        _install_ntff_hook()
    res = run_bass_kernel_spmd(nc, in_maps, core_ids=list(range(NCORES)),
                               trace=trace,
                               trace_cores=list(range(NCORES)) if trace else None)
    LAST_EXEC_TIME_NS = res.exec_time_ns
    LAST_RESULTS = res
    return np.concatenate(
        [res.results[c]["out"][:, 0] for c in range(NCORES)]).astype(np.float32)
